# revision 48
# baseline (speedup 1.0000x reference)
"""Trainium2 Bass kernel: 12-head attention block (qkv proj -> softmax attn -> fc).

Reference semantics (B=32, S=577, D=768, H=12, Dh=64):
    qkv = x @ w_qkv + b_qkv
    q, k, v = split(qkv); attn = softmax(q k^T / 8) v
    out = attn @ w_fc + b_fc

Sharding: data-parallel over batch across 8 NeuronCores (4 images per core),
weights replicated, no collectives. Compute in bf16 with fp32 PSUM accumulation.

Layout (all matmuls contract over the partition dim):
  - xT [768, 577] per image: x lands f32 via the two HWDGE queues (each
    s-tile split into D-halves across both), one DVE cast to bf16 per
    s-tile, then PE transposes in bf16 (1 cyc/col + fast weight load; the
    old fp32 transpose-mode path ran at 1/4 rate).
  - qkT [1536, 577] = w_qkv[:, :1536]^T . xT; per-partition qkv bias is
    loaded as a contiguous [12, 128] block and PE-transposed (the direct
    [p, m] gather is 1536 4B descriptors and camps on the sync queue).
  - v [577, 768] + per-head ones column so attention row-sums fall out of
    the attn@v matmul for free.
  - scoresT[sk, sq] = kT_h^T . qT_h; head pairs on disjoint PE row groups;
    exp on ScalarE (scale folded in; both chunks of a head emitted
    back-to-back so its psum bank frees as early as possible).
  - attn_outT[65, sq]: row 64 = softmax denominators; normalize =
    rs copy -> reciprocal_approx_fast (NOT from PSUM: silently wrong on HW)
    -> gpsimd partition_broadcast [64 rows] -> fused DVE multiply straight
    from PSUM into attnT (saves a full unnormalized-copy pass).
  - fc: out[s, :] = attnT_k^T . w_fc_k; per-(si, 512/256-chunk) units with
    per-chunk output DMAs round-robined over both HWDGE queues.

Scheduling (the part that matters): the attention phase is paced by exp on
ScalarE (~1.06us per scores si-group vs ~0.45us of PE), so all other PE
work -- qkT half-chains, v/fc chunks, next image's transposes -- is kept in
a deadline-ordered queue of ~0.7-1.3us units and injected between scores
si-groups by a quota + exp-deficit scheduler.  Sub-us PE idle gaps are not
just lost time: enough of them in a 3.4us window re-throttles the PE HAM
clock gate to half rate, so even spreading matters more than total filler
supply (the old one-filler-per-si-group scheme starved pairs 3-5 and ran
13-20us of every batch at 1.2GHz).  Every psum tile is a single 1-bank
chunk (psW pool bufs=4) so a chain's first matmul only waits on a DVE
drain 4 allocations back.  Weight DMAs are contiguous row-blocks only
(SWDGE casting throughput collapses ~7x on column-sliced reads): q/k half
of k0-3 + v half + w_fc on the SWDGE casting queue, q/k of k4-5 staged f32
on the idle HWDGE queues and DVE-cast, so scores pair 0 issues ~5us
earlier than a pure-SWDGE stream allows.
PSUM: psS (scores) 2x2-bank slots, psW (everything else) 4x1-bank slots.
"""

import os
import sys

import numpy as np

for _p in ("/opt/trn_rl_repo", "/root/.axon_site/_ro/trn_rl_repo"):
    if os.path.isdir(_p) and _p not in sys.path:
        sys.path.insert(0, _p)

import concourse.bass as bass  # noqa: E402
import concourse.tile as tile  # noqa: E402
from concourse import bacc, mybir  # noqa: E402
from concourse.bass_utils import run_bass_kernel_spmd  # noqa: E402
from concourse.masks import make_identity  # noqa: E402

F32 = mybir.dt.float32
BF16 = mybir.dt.bfloat16

B, S, D = 32, 577, 768
H, DH = 12, 64
NCORES = 8
NB = B // NCORES  # 4 batch images per core
SCALE = DH**-0.5  # 0.125
NKT = D // 128  # 6 contraction tiles of 128
S_TILES = [(0, 128), (128, 128), (256, 128), (384, 128), (512, 65)]
CH_S = [(0, 512), (512, 65)]  # 577 split at PSUM-bank boundary
CH_D = [(0, 512), (512, 256)]  # 768 split at PSUM-bank boundary
SPAD = 640  # xT column allocation (577 rounded up to 5*128)
EXP = mybir.ActivationFunctionType.Exp
IDENT = mybir.ActivationFunctionType.Identity


def build_nc():
    nc = bacc.Bacc(None)
    x_ext = nc.declare_dram_parameter("x", [NB, S, D], F32, isOutput=False)
    wqkv_ext = nc.declare_dram_parameter("w_qkv", [D, 3 * D], F32, isOutput=False)
    bqkv_ext = nc.declare_dram_parameter("b_qkv", [3 * D], F32, isOutput=False)
    wfc_ext = nc.declare_dram_parameter("w_fc", [D, D], F32, isOutput=False)
    bfc_ext = nc.declare_dram_parameter("b_fc", [D], F32, isOutput=False)
    out_ext = nc.declare_dram_parameter("out", [NB, S, D], F32, isOutput=True)

    with tile.TileContext(nc) as tc:
        with (
            tc.tile_pool(name="const", bufs=1) as cpool,
            tc.tile_pool(name="xf", bufs=1) as xf_pool,
            tc.tile_pool(name="xb", bufs=1) as xb_pool,
            tc.tile_pool(name="xT", bufs=2) as xT_pool,
            tc.tile_pool(name="qkT", bufs=2) as qkT_pool,
            tc.tile_pool(name="v", bufs=2) as v_pool,
            tc.tile_pool(name="expT", bufs=4) as expT_pool,
            tc.tile_pool(name="attnT", bufs=2) as attnT_pool,
            tc.tile_pool(name="small", bufs=3) as small_pool,
            tc.tile_pool(name="osb", bufs=3) as osb_pool,
            tc.tile_pool(name="psS", bufs=2, space="PSUM") as psS,
            tc.tile_pool(name="psW", bufs=4, space="PSUM") as psW,
        ):
            # ---- small constants first on the sync HWDGE (x0 right behind) ----
            ones = cpool.tile([1, 128], F32)
            nc.vector.memset(ones[:], 1.0)
            ones16 = cpool.tile([1, 128], BF16)
            nc.vector.memset(ones16[:], 1.0)
            identity = cpool.tile([128, 128], BF16)
            make_identity(nc, identity[:])
            brow_v = cpool.tile([1, D], F32)
            nc.sync.dma_start(brow_v[:], bqkv_ext[None, 2 * D : 3 * D])
            brow_fc = cpool.tile([1, D], F32)
            nc.scalar.dma_start(brow_fc[:], bfc_ext[None, :])
            brow_fc16 = cpool.tile([1, D], BF16)
            nc.vector.tensor_copy(brow_fc16[:], brow_fc[:])

            # ---- x pipeline: f32 DMA (HWDGE) -> DVE cast -> XBAR transpose ----
            xf_t, xb_t, xT_t = {}, {}, {}

            def emit_x_dma(b):
                xf_t[b] = xf_pool.tile([128, 5 * D], F32, tag="xf", name="xf")
                for si, (s0, psl) in enumerate(S_TILES):
                    eng = nc.sync if si % 2 == 0 else nc.scalar
                    eng.dma_start(
                        xf_t[b][0:psl, si * D : (si + 1) * D],
                        x_ext[b, s0 : s0 + psl, :],
                    )

            def emit_xT_cast(b):
                # cast each s-tile to bf16 (one DVE copy per s-tile)
                xb_t[b] = xb_pool.tile([128, 5 * D], BF16, tag="xb", name="xb")
                xT_t[b] = {}
                for si, (s0, psl) in enumerate(S_TILES):
                    nc.vector.tensor_copy(
                        xb_t[b][0:psl, si * D : (si + 1) * D],
                        xf_t[b][0:psl, si * D : (si + 1) * D],
                    )

            def emit_xT_dk(b, dk):
                # PE-transpose one 128-row k-tile in bf16 (1 cyc/col + fast
                # weight load, vs 1/4 rate in f32)
                xT_t[b][dk] = xT_pool.tile(
                    [128, S], BF16, tag=f"xT{dk}", name=f"xT{dk}"
                )
                px = psW.tile([128, S], BF16, tag="psW", name="px")
                for si, (s0, psl) in enumerate(S_TILES):
                    nc.tensor.transpose(
                        px[:, s0 : s0 + psl],
                        xb_t[b][0:psl, si * D + dk * 128 : si * D + (dk + 1) * 128],
                        identity[0:psl, 0:psl],
                    )
                nc.vector.tensor_copy(xT_t[b][dk][:], px[:])

            def emit_xT(b):
                emit_xT_cast(b)
                for dk in range(NKT):
                    emit_xT_dk(b, dk)

            # ---- weight tiles (SBUF, bf16, SWDGE casting DMAs) ----
            # contiguous row-block DMAs only: SWDGE casting throughput
            # collapses ~7x on column-sliced (strided) reads
            w_qkv_k = [
                cpool.tile([128, 3 * D], BF16, name=f"wqkv{k}") for k in range(NKT)
            ]
            w_fc_k = [cpool.tile([128, D], BF16, name=f"wfc{k}") for k in range(NKT)]
            for k in range(4):
                # q/k half for k4,k5 arrives via HWDGE f32 + DVE cast below
                nc.gpsimd.dma_start(
                    w_qkv_k[k][:, 0 : 2 * D],
                    wqkv_ext[k * 128 : (k + 1) * 128, 0 : 2 * D],
                )
            for k in range(NKT):
                nc.gpsimd.dma_start(
                    w_qkv_k[k][:, 2 * D : 3 * D],
                    wqkv_ext[k * 128 : (k + 1) * 128, 2 * D : 3 * D],
                )
            for k in range(NKT):
                nc.gpsimd.dma_start(w_fc_k[k][:], wfc_ext[k * 128 : (k + 1) * 128, :])

            # per-partition bias for qkT tiles: load [12, 128] contiguous and
            # PE-transpose (the direct [p, m] gather is 1536 4B descriptors
            # and occupies the sync queue for ~30us)
            bqk_nat = cpool.tile([12, 128], F32)
            nc.sync.dma_start(
                bqk_nat[:], bqkv_ext[0 : 2 * D].rearrange("(m p) -> m p", p=128)
            )
            bqk_nat16 = cpool.tile([12, 128], BF16)
            nc.vector.tensor_copy(bqk_nat16[:], bqk_nat[:])
            b_qk = cpool.tile([128, H], F32)  # [p, m]

            # x0: split every s-tile into D-halves across both HWDGE queues
            xf_t[0] = xf_pool.tile([128, 5 * D], F32, tag="xf", name="xf")
            for si, (s0, psl) in enumerate(S_TILES):
                for hi, eng in ((0, nc.sync), (1, nc.scalar)):
                    eng.dma_start(
                        xf_t[0][0:psl, si * D + hi * 384 : si * D + (hi + 1) * 384],
                        x_ext[0, s0 : s0 + psl, hi * 384 : (hi + 1) * 384],
                    )
            emit_xT_cast(0)


            # qk-half k4,k5: contiguous f32 row-blocks on the (otherwise
            # idle) HWDGE queues + DVE cast (emitted after the xT copies so
            # the cast doesn't block the transpose psum-slot drains) -- takes
            # 2 of the 6 k-tiles off the serial SWDGE stream so scores pair 0
            # starts ~5us earlier
            with tc.tile_pool(name="wqstage", bufs=1) as wqs:
                wq_st = {}
                for i, k in enumerate((4, 5)):
                    wq_st[k] = wqs.tile(
                        [128, 2 * D], F32, tag=f"wq{i}", name=f"wq{i}"
                    )
                    eng = nc.sync if i == 0 else nc.scalar
                    eng.dma_start(
                        wq_st[k][:], wqkv_ext[k * 128 : (k + 1) * 128, 0 : 2 * D]
                    )

                for dk in range(NKT):
                    emit_xT_dk(0, dk)

                # broadcast v/fc biases to all 128 partitions via K=1 matmul
                b_v_bc = cpool.tile([128, D], F32)
                b_fc_bc = cpool.tile([128, D], F32)
                for row, bc in ((brow_v, b_v_bc), (brow_fc, b_fc_bc)):
                    for c0, cl in CH_D:
                        pb = psW.tile([128, 512], F32, tag="psW", name="pb")
                        nc.tensor.matmul(
                            pb[:, 0:cl],
                            lhsT=ones[0:1, 0:128],
                            rhs=row[0:1, c0 : c0 + cl],
                            start=True,
                            stop=True,
                        )
                        nc.vector.tensor_copy(bc[:, c0 : c0 + cl], pb[:, 0:cl])

                pbq = psW.tile([128, H], BF16, tag="psW", name="pbq")
                nc.tensor.transpose(
                    pbq[:, 0:H], bqk_nat16[0:H, 0:128], identity[0:H, 0:H]
                )
                nc.vector.tensor_copy(b_qk[:], pbq[:, 0:H])

                for k in (4, 5):
                    nc.vector.tensor_copy(w_qkv_k[k][:, 0 : 2 * D], wq_st[k][:])
            emit_x_dma(1)

            # ---- compute emission helpers ----
            qkT_t = {}

            qkT_pend = {}

            def emit_qkT_half(b, m, half):
                if half == 0:
                    if b not in qkT_t:
                        qkT_t[b] = {}
                    qkT_t[b][m] = qkT_pool.tile(
                        [128, S], BF16, tag=f"qkT{m}", name=f"qkT{m}"
                    )
                    qkT_pend[(b, m)] = {
                        0: psW.tile([128, 512], F32, tag="psW", name="pqk5"),
                        512: psW.tile([128, 65], F32, tag="psW", name="pqk65"),
                    }
                pqk = qkT_pend[(b, m)]
                ks = range(0, 3) if half == 0 else range(3, NKT)
                for k in ks:
                    for c0, cl in CH_S:
                        nc.tensor.matmul(
                            pqk[c0][:, 0:cl],
                            lhsT=w_qkv_k[k][:, m * 128 : (m + 1) * 128],
                            rhs=xT_t[b][k][:, c0 : c0 + cl],
                            start=(k == 0),
                            stop=(k == NKT - 1),
                        )
                if half == 1:
                    for c0, cl in CH_S:
                        nc.scalar.activation(
                            qkT_t[b][m][:, c0 : c0 + cl], pqk[c0][:, 0:cl],
                            IDENT, bias=b_qk[:, m : m + 1],
                        )
                    del qkT_pend[(b, m)]

            def emit_qkT_mtile(b, m):
                emit_qkT_half(b, m, 0)
                emit_qkT_half(b, m, 1)

            v_t = {}

            def emit_v_chunk(b, si, c0, cl):
                if b not in v_t:
                    v_t[b] = v_pool.tile(
                        [128, 5 * H * (DH + 1)], BF16, tag="v", name="v_all"
                    )
                    v4m = v_t[b][:].rearrange("p (s h e) -> p s h e", s=5, h=H)
                    nc.vector.memset(v4m[:, :, :, DH : DH + 1], 1.0)
                s0, psl = S_TILES[si]
                v4 = v_t[b][:].rearrange("p (s h e) -> p s h e", s=5, h=H)
                pv = psW.tile([128, cl], F32, tag="psW", name=f"pv{cl}")
                for k in range(NKT):
                    nc.tensor.matmul(
                        pv[0:psl, 0:cl],
                        lhsT=xT_t[b][k][:, s0 : s0 + psl],
                        rhs=w_qkv_k[k][:, 2 * D + c0 : 2 * D + c0 + cl],
                        start=(k == 0),
                        stop=(k == NKT - 1),
                    )
                h0, hn = c0 // DH, cl // DH
                nc.vector.tensor_add(
                    v4[0:psl, si, h0 : h0 + hn, 0:DH],
                    pv[0:psl, 0:cl].rearrange("p (h e) -> p h e", h=hn),
                    b_v_bc[0:psl, c0 : c0 + cl].rearrange("p (h e) -> p h e", h=hn),
                )

            def emit_v_si(b, si):
                for c0, cl in CH_D:
                    emit_v_chunk(b, si, c0, cl)

            def emit_scores_si(b, p, si, expT):
                heads = (2 * p, 2 * p + 1)
                s0, psl = S_TILES[si]
                psc = {}
                for h in heads:
                    if si == 0:
                        expT[h] = expT_pool.tile(
                            [128, 5 * S], BF16, tag="expT", name=f"expT{h % 2}"
                        )
                    psc[h] = psS.tile([128, S], F32, tag="psS", name=f"psc{h % 2}")
                for h in heads:
                    hoff = (h % 2) * 64
                    qm, km = h // 2, NKT + h // 2
                    for c0, cl in CH_S:
                        nc.tensor.matmul(
                            psc[h][0:psl, c0 : c0 + cl],
                            lhsT=qkT_t[b][km][hoff : hoff + 64, s0 : s0 + psl],
                            rhs=qkT_t[b][qm][hoff : hoff + 64, c0 : c0 + cl],
                            start=True,
                            stop=True,
                        )
                for h in heads:
                    nc.scalar.activation(
                        expT[h][0:psl, si * S : (si + 1) * S],
                        psc[h][0:psl, :],
                        EXP,
                        scale=float(SCALE),
                    )

            def emit_attnv_head(b, h, attnT_all, expT):
                hoff = (h % 2) * 64
                # attn_outT [65, 577]: rows 0:64 = out^T unnorm, row 64 = sums
                po = {0: psW.tile([65, 512], F32, tag="psW", name="po5"),
                      512: psW.tile([65, 65], F32, tag="psW", name="po65")}
                for si, (s0, psl) in enumerate(S_TILES):
                    for c0, cl in CH_S:
                        nc.tensor.matmul(
                            po[c0][:, 0:cl],
                            lhsT=v_t[b][
                                0:psl,
                                si * H * (DH + 1)
                                + h * (DH + 1) : si * H * (DH + 1)
                                + (h + 1) * (DH + 1),
                            ],
                            rhs=expT[h][0:psl, si * S + c0 : si * S + c0 + cl],
                            start=(si == 0),
                            stop=(si == 4),
                        )
                # normalize: sums row to SBUF (reciprocal can't read PSUM on
                # HW), reciprocal, broadcast, fused PSUM multiply
                rs = small_pool.tile([1, S], F32, tag="rs", name=f"rs{h % 2}")
                for c0, cl in CH_S:
                    nc.vector.tensor_copy(rs[:, c0 : c0 + cl], po[c0][64:65, 0:cl])
                rinv = small_pool.tile([1, S], F32, tag="rinv", name=f"rinv{h % 2}")
                nc.vector.reciprocal_approx_fast(rinv[:], rs[:])
                rbc = small_pool.tile([64, S], F32, tag="rbc")
                nc.gpsimd.partition_broadcast(rbc[:, :], rinv[0:1, :], channels=64)
                for c0, cl in CH_S:
                    nc.vector.tensor_mul(
                        attnT_all[
                            hoff : hoff + 64, (h // 2) * S + c0 : (h // 2) * S + c0 + cl
                        ],
                        po[c0][0:64, 0:cl],
                        rbc[:, c0 : c0 + cl],
                    )
                del expT[h]

            def emit_fc_chunk(b, si, c0, cl):
                s0, psl = S_TILES[si]
                attnT_all = attnT_t[b]
                pf = psW.tile([128, cl], F32, tag="psW", name=f"pf{cl}")
                for k in range(NKT):
                    nc.tensor.matmul(
                        pf[0:psl, 0:cl],
                        lhsT=attnT_all[:, k * S + s0 : k * S + s0 + psl],
                        rhs=w_fc_k[k][:, c0 : c0 + cl],
                        start=(k == 0),
                        stop=(k == NKT - 1),
                    )
                osb = osb_pool.tile([128, cl], F32, tag="osb")
                nc.vector.tensor_add(
                    osb[0:psl, 0:cl], pf[0:psl, 0:cl], b_fc_bc[0:psl, c0 : c0 + cl]
                )
                eng = nc.sync if (si + c0 // 512) % 2 else nc.scalar
                eng.dma_start(
                    out_ext[b, s0 : s0 + psl, c0 : c0 + cl], osb[0:psl, 0:cl]
                )

            def emit_fc_si(b, si):
                for c0, cl in CH_D:
                    emit_fc_chunk(b, si, c0, cl)

            def emit_fc_chunk_tail(b, si, c0, cl):
                # final-batch variant: fc bias lands via a cheap K=1 bf16
                # ones-matmul and the psum drains through ScalarE (idle after
                # the last exp) -- no DVE dependency, so the tail fc chains
                # don't stall into a HAM re-throttle
                s0, psl = S_TILES[si]
                attnT_all = attnT_t[b]
                pf = psW.tile([128, cl], F32, tag="psW", name=f"pf{cl}")
                nc.tensor.matmul(
                    pf[0:psl, 0:cl],
                    lhsT=ones16[0:1, 0:psl],
                    rhs=brow_fc16[0:1, c0 : c0 + cl],
                    start=True,
                    stop=False,
                )
                for k in range(NKT):
                    nc.tensor.matmul(
                        pf[0:psl, 0:cl],
                        lhsT=attnT_all[:, k * S + s0 : k * S + s0 + psl],
                        rhs=w_fc_k[k][:, c0 : c0 + cl],
                        start=False,
                        stop=(k == NKT - 1),
                    )
                osb = osb_pool.tile([128, cl], F32, tag="osb")
                nc.scalar.activation(osb[0:psl, 0:cl], pf[0:psl, 0:cl], IDENT)
                eng = nc.sync if (si + c0 // 512) % 2 else nc.scalar
                eng.dma_start(
                    out_ext[b, s0 : s0 + psl, c0 : c0 + cl], osb[0:psl, 0:cl]
                )

            # ---- prologue PE work: just enough to reach scores pair 0 ----
            emit_qkT_mtile(0, 0)
            emit_qkT_mtile(0, 6)

            # ---- deficit-tracked filler scheduler with deadline forcing ----
            PE_SCORES_SI = 0.45
            PE_ATTNV_HEAD = 1.30
            SC_EXP_SI = 1.06
            state = {"pe": 0.0, "sc": 0.0, "spent": 0.0}
            fill_q = []  # list of [key, cost_us, fn]

            def emit_unit(i):
                key, cost, fn = fill_q.pop(i)
                fn()
                state["pe"] += cost
                state["spent"] += cost

            def force(*keys):
                # emit every queued unit whose key prefix-matches (both
                # halves of a split unit), in queue order
                for key in keys:
                    while True:
                        idx = next(
                            (i for i, e in enumerate(fill_q)
                             if e[0][: len(key)] == key),
                            None,
                        )
                        if idx is None:
                            break
                        emit_unit(idx)

            attnT_t = {}
            for b in range(NB):
                if b + 2 < NB:
                    emit_x_dma(b + 2)
                # this batch's filler queue in DEADLINE order: this batch's
                # qkT, then next batch's qkT/v/xT (their DVE tails must land
                # before the next batch starts), fc last (no deadline)
                def u_qkT(bb, mm):
                    return [
                        [("qkT", bb, mm, 0), 0.72,
                         (lambda: emit_qkT_half(bb, mm, 0))],
                        [("qkT", bb, mm, 1), 0.73,
                         (lambda: emit_qkT_half(bb, mm, 1))],
                    ]

                def u_v(bb, ss):
                    return [
                        [("v", bb, ss, 0), 1.30,
                         (lambda: emit_v_chunk(bb, ss, 0, 512))],
                        [("v", bb, ss, 1), 0.66,
                         (lambda: emit_v_chunk(bb, ss, 512, 256))],
                    ]

                def u_fc(bb, ss):
                    return [
                        [("fc", bb, ss, 0), 1.35,
                         (lambda: emit_fc_chunk(bb, ss, 0, 512))],
                        [("fc", bb, ss, 1), 0.68,
                         (lambda: emit_fc_chunk(bb, ss, 512, 256))],
                    ]

                fq = []
                if b == 0:
                    fq.append([("xTc", 1), 0.0, lambda: emit_xT_cast(1)])
                    for dk in range(NKT):
                        fq.append(
                            [("xT", 1, dk), 0.75,
                             (lambda kk: lambda: emit_xT_dk(1, kk))(dk)]
                        )
                    for m in (1, 7):
                        fq.extend(u_qkT(0, m))
                    for si in range(5):
                        fq.extend(u_v(0, si))
                for p in range(2, NKT):
                    for m in (p, NKT + p):
                        fq.extend(u_qkT(b, m))
                if b + 1 < NB:
                    for m in (0, 6, 1, 7):
                        fq.extend(u_qkT(b + 1, m))
                    for si in range(5):
                        fq.extend(u_v(b + 1, si))
                if b + 2 < NB:
                    fq.append(
                        [("xTc", b + 2), 0.0,
                         (lambda bb: lambda: emit_xT_cast(bb))(b + 2)]
                    )
                    for dk in range(NKT):
                        fq.append(
                            [("xT", b + 2, dk), 0.75,
                             (lambda bb, kk: lambda: emit_xT_dk(bb, kk))(b + 2, dk)]
                        )
                if b >= 1:
                    for si in range(5):
                        fq.extend(u_fc(b - 1, si))
                fill_q = fq
                total_cost = sum(e[1] for e in fq)
                state["pe"] = 0.0
                state["sc"] = 0.0
                state["spent"] = 0.0
                point = [0]
                N_POINTS = 5 * (H // 2) + H  # 30 si-groups + 12 attnv heads

                def quota_fill():
                    # guaranteed even spreading: by point i, (i+1)/N of the
                    # queue's cost must be emitted, with at least one unit per
                    # point (sub-us stalls trip the HAM clock-gate window);
                    # plus the exp-deficit floor
                    point[0] += 1
                    tgt = total_cost * point[0] / N_POINTS
                    first = len(fill_q) > (N_POINTS - point[0])
                    while fill_q and (
                        first
                        or state["spent"] < tgt
                        or state["pe"] < state["sc"] + 0.4
                    ):
                        first = False
                        emit_unit(0)

                attnT_t[b] = attnT_pool.tile(
                    [128, NKT * S], BF16, tag="attnT", name="attnT_all"
                )
                # deadline safety: v + first qkT tiles must exist before use
                if b > 0:
                    force(("qkT", b, 0), ("qkT", b, 6), ("qkT", b, 1), ("qkT", b, 7),
                          *((("v", b, si)) for si in range(5)))
                expT = {}
                for p in range(H // 2 + 1):
                    if p < H // 2:
                        force(("qkT", b, p), ("qkT", b, NKT + p))
                        for si in range(5):
                            emit_scores_si(b, p, si, expT)
                            state["pe"] += PE_SCORES_SI
                            state["sc"] += SC_EXP_SI
                            quota_fill()
                    if p >= 1:
                        if p == 1:
                            force(*[("v", b, si) for si in range(5)])
                        for h in (2 * (p - 1), 2 * (p - 1) + 1):
                            emit_attnv_head(b, h, attnT_t[b], expT)
                            state["pe"] += PE_ATTNV_HEAD
                            quota_fill()
                # drain leftovers (should be nearly empty now)
                while fill_q:
                    emit_unit(0)

            for si in range(5):
                for c0, cl in CH_D:
                    emit_fc_chunk_tail(NB - 1, si, c0, cl)

    nc.compile()
    return nc


_NC_CACHE = None


def _get_nc():
    global _NC_CACHE
    if _NC_CACHE is None:
        _NC_CACHE = build_nc()
    return _NC_CACHE


def kernel(x, w_qkv, b_qkv, w_fc, b_fc, _collect=None):
    nc = _get_nc()
    x = np.ascontiguousarray(np.asarray(x, dtype=np.float32))
    w_qkv = np.ascontiguousarray(np.asarray(w_qkv, dtype=np.float32))
    b_qkv = np.ascontiguousarray(np.asarray(b_qkv, dtype=np.float32))
    w_fc = np.ascontiguousarray(np.asarray(w_fc, dtype=np.float32))
    b_fc = np.ascontiguousarray(np.asarray(b_fc, dtype=np.float32))
    in_maps = [
        {
            "x": x[i * NB : (i + 1) * NB],
            "w_qkv": w_qkv,
            "b_qkv": b_qkv,
            "w_fc": w_fc,
            "b_fc": b_fc,
        }
        for i in range(NCORES)
    ]
    kwargs = dict(_collect) if _collect else {}
    res = run_bass_kernel_spmd(nc, in_maps, core_ids=list(range(NCORES)), **kwargs)
    out = np.concatenate([res.results[i]["out"] for i in range(NCORES)], axis=0)
    if _collect is not None and isinstance(_collect, dict):
        _collect["result"] = res
    return out.astype(np.float32)


if __name__ == "__main__":
    xs = np.random.randn(B, S, D).astype(np.float32)
    lim = 1.0 / np.sqrt(D)
    rng = np.random.default_rng(0)
    wq = rng.uniform(-lim, lim, (D, 3 * D)).astype(np.float32)
    bq = rng.uniform(-lim, lim, (3 * D,)).astype(np.float32)
    wf = rng.uniform(-lim, lim, (D, D)).astype(np.float32)
    bf = rng.uniform(-lim, lim, (D,)).astype(np.float32)
    o = kernel(xs, wq, bq, wf, bf)
    print("out", o.shape, o.dtype)


# revision 49
# speedup vs baseline: 1.0019x; 1.0019x over previous
"""Trainium2 Bass kernel: 12-head attention block (qkv proj -> softmax attn -> fc).

Reference semantics (B=32, S=577, D=768, H=12, Dh=64):
    qkv = x @ w_qkv + b_qkv
    q, k, v = split(qkv); attn = softmax(q k^T / 8) v
    out = attn @ w_fc + b_fc

Sharding: data-parallel over batch across 8 NeuronCores (4 images per core),
weights replicated, no collectives. Compute in bf16 with fp32 PSUM accumulation.

Layout (all matmuls contract over the partition dim):
  - xT [768, 577] per image: x lands f32 via the two HWDGE queues (each
    s-tile split into D-halves across both), one DVE cast to bf16 per
    s-tile, then PE transposes in bf16 (1 cyc/col + fast weight load; the
    old fp32 transpose-mode path ran at 1/4 rate).
  - qkT [1536, 577] = w_qkv[:, :1536]^T . xT; per-partition qkv bias is
    loaded as a contiguous [12, 128] block and PE-transposed (the direct
    [p, m] gather is 1536 4B descriptors and camps on the sync queue).
  - v [577, 768] + per-head ones column so attention row-sums fall out of
    the attn@v matmul for free.
  - scoresT[sk, sq] = kT_h^T . qT_h; head pairs on disjoint PE row groups;
    exp on ScalarE (scale folded in; both chunks of a head emitted
    back-to-back so its psum bank frees as early as possible).
  - attn_outT[65, sq]: row 64 = softmax denominators; normalize =
    rs copy -> reciprocal_approx_fast (NOT from PSUM: silently wrong on HW)
    -> gpsimd partition_broadcast [64 rows] -> fused DVE multiply straight
    from PSUM into attnT (saves a full unnormalized-copy pass).
  - fc: out[s, :] = attnT_k^T . w_fc_k; per-(si, 512/256-chunk) units with
    per-chunk output DMAs round-robined over both HWDGE queues.

Scheduling (the part that matters): the attention phase is paced by exp on
ScalarE (~1.06us per scores si-group vs ~0.45us of PE), so all other PE
work -- qkT half-chains, v/fc chunks, next image's transposes -- is kept in
a deadline-ordered queue of ~0.7-1.3us units and injected between scores
si-groups by a quota + exp-deficit scheduler.  Sub-us PE idle gaps are not
just lost time: enough of them in a 3.4us window re-throttles the PE HAM
clock gate to half rate, so even spreading matters more than total filler
supply (the old one-filler-per-si-group scheme starved pairs 3-5 and ran
13-20us of every batch at 1.2GHz).  Every psum tile is a single 1-bank
chunk (psW pool bufs=4) so a chain's first matmul only waits on a DVE
drain 4 allocations back.  Weight DMAs are contiguous row-blocks only
(SWDGE casting throughput collapses ~7x on column-sliced reads): q/k half
of k0-3 + v half + w_fc on the SWDGE casting queue, q/k of k4-5 staged f32
on the idle HWDGE queues and DVE-cast, so scores pair 0 issues ~5us
earlier than a pure-SWDGE stream allows.
PSUM: psS (scores) 2x2-bank slots, psW (everything else) 4x1-bank slots.
"""

import os
import sys

import numpy as np

for _p in ("/opt/trn_rl_repo", "/root/.axon_site/_ro/trn_rl_repo"):
    if os.path.isdir(_p) and _p not in sys.path:
        sys.path.insert(0, _p)

import concourse.bass as bass  # noqa: E402
import concourse.tile as tile  # noqa: E402
from concourse import bacc, mybir  # noqa: E402
from concourse.bass_utils import run_bass_kernel_spmd  # noqa: E402
from concourse.masks import make_identity  # noqa: E402

F32 = mybir.dt.float32
BF16 = mybir.dt.bfloat16

B, S, D = 32, 577, 768
H, DH = 12, 64
NCORES = 8
NB = B // NCORES  # 4 batch images per core
SCALE = DH**-0.5  # 0.125
NKT = D // 128  # 6 contraction tiles of 128
S_TILES = [(0, 128), (128, 128), (256, 128), (384, 128), (512, 65)]
CH_S = [(0, 512), (512, 65)]  # 577 split at PSUM-bank boundary
CH_D = [(0, 512), (512, 256)]  # 768 split at PSUM-bank boundary
SPAD = 640  # xT column allocation (577 rounded up to 5*128)
EXP = mybir.ActivationFunctionType.Exp
IDENT = mybir.ActivationFunctionType.Identity


def build_nc():
    nc = bacc.Bacc(None)
    x_ext = nc.declare_dram_parameter("x", [NB, S, D], F32, isOutput=False)
    wqkv_ext = nc.declare_dram_parameter("w_qkv", [D, 3 * D], F32, isOutput=False)
    bqkv_ext = nc.declare_dram_parameter("b_qkv", [3 * D], F32, isOutput=False)
    wfc_ext = nc.declare_dram_parameter("w_fc", [D, D], F32, isOutput=False)
    bfc_ext = nc.declare_dram_parameter("b_fc", [D], F32, isOutput=False)
    out_ext = nc.declare_dram_parameter("out", [NB, S, D], F32, isOutput=True)

    with tile.TileContext(nc) as tc:
        with (
            tc.tile_pool(name="const", bufs=1) as cpool,
            tc.tile_pool(name="xf", bufs=1) as xf_pool,
            tc.tile_pool(name="xb", bufs=1) as xb_pool,
            tc.tile_pool(name="xT", bufs=2) as xT_pool,
            tc.tile_pool(name="qkT", bufs=2) as qkT_pool,
            tc.tile_pool(name="v", bufs=2) as v_pool,
            tc.tile_pool(name="expT", bufs=4) as expT_pool,
            tc.tile_pool(name="attnT", bufs=2) as attnT_pool,
            tc.tile_pool(name="small", bufs=3) as small_pool,
            tc.tile_pool(name="osb", bufs=3) as osb_pool,
            tc.tile_pool(name="psS", bufs=2, space="PSUM") as psS,
            tc.tile_pool(name="psW", bufs=4, space="PSUM") as psW,
        ):
            # ---- small constants first on the sync HWDGE (x0 right behind) ----
            ones = cpool.tile([1, 128], F32)
            nc.vector.memset(ones[:], 1.0)
            identity = cpool.tile([128, 128], BF16)
            make_identity(nc, identity[:])
            brow_v = cpool.tile([1, D], F32)
            nc.sync.dma_start(brow_v[:], bqkv_ext[None, 2 * D : 3 * D])
            brow_fc = cpool.tile([1, D], F32)
            nc.scalar.dma_start(brow_fc[:], bfc_ext[None, :])

            # ---- x pipeline: f32 DMA (HWDGE) -> DVE cast -> XBAR transpose ----
            xf_t, xb_t, xT_t = {}, {}, {}

            def emit_x_dma(b):
                xf_t[b] = xf_pool.tile([128, 5 * D], F32, tag="xf", name="xf")
                for si, (s0, psl) in enumerate(S_TILES):
                    eng = nc.sync if si % 2 == 0 else nc.scalar
                    eng.dma_start(
                        xf_t[b][0:psl, si * D : (si + 1) * D],
                        x_ext[b, s0 : s0 + psl, :],
                    )

            def emit_xT_cast(b):
                # cast each s-tile to bf16 (one DVE copy per s-tile)
                xb_t[b] = xb_pool.tile([128, 5 * D], BF16, tag="xb", name="xb")
                xT_t[b] = {}
                for si, (s0, psl) in enumerate(S_TILES):
                    nc.vector.tensor_copy(
                        xb_t[b][0:psl, si * D : (si + 1) * D],
                        xf_t[b][0:psl, si * D : (si + 1) * D],
                    )

            def emit_xT_dk(b, dk):
                # PE-transpose one 128-row k-tile in bf16 (1 cyc/col + fast
                # weight load, vs 1/4 rate in f32)
                xT_t[b][dk] = xT_pool.tile(
                    [128, S], BF16, tag=f"xT{dk}", name=f"xT{dk}"
                )
                px = psW.tile([128, S], BF16, tag="psW", name="px")
                for si, (s0, psl) in enumerate(S_TILES):
                    nc.tensor.transpose(
                        px[:, s0 : s0 + psl],
                        xb_t[b][0:psl, si * D + dk * 128 : si * D + (dk + 1) * 128],
                        identity[0:psl, 0:psl],
                    )
                nc.vector.tensor_copy(xT_t[b][dk][:], px[:])

            def emit_xT(b):
                emit_xT_cast(b)
                for dk in range(NKT):
                    emit_xT_dk(b, dk)

            # ---- weight tiles (SBUF, bf16, SWDGE casting DMAs) ----
            # contiguous row-block DMAs only: SWDGE casting throughput
            # collapses ~7x on column-sliced (strided) reads
            w_qkv_k = [
                cpool.tile([128, 3 * D], BF16, name=f"wqkv{k}") for k in range(NKT)
            ]
            w_fc_k = [cpool.tile([128, D], BF16, name=f"wfc{k}") for k in range(NKT)]
            for k in range(4):
                # q/k half for k4,k5 arrives via HWDGE f32 + DVE cast below
                nc.gpsimd.dma_start(
                    w_qkv_k[k][:, 0 : 2 * D],
                    wqkv_ext[k * 128 : (k + 1) * 128, 0 : 2 * D],
                )
            for k in range(NKT):
                nc.gpsimd.dma_start(
                    w_qkv_k[k][:, 2 * D : 3 * D],
                    wqkv_ext[k * 128 : (k + 1) * 128, 2 * D : 3 * D],
                )
            for k in range(NKT):
                nc.gpsimd.dma_start(w_fc_k[k][:], wfc_ext[k * 128 : (k + 1) * 128, :])

            # per-partition bias for qkT tiles: load [12, 128] contiguous and
            # PE-transpose (the direct [p, m] gather is 1536 4B descriptors
            # and occupies the sync queue for ~30us)
            bqk_nat = cpool.tile([12, 128], F32)
            nc.sync.dma_start(
                bqk_nat[:], bqkv_ext[0 : 2 * D].rearrange("(m p) -> m p", p=128)
            )
            bqk_nat16 = cpool.tile([12, 128], BF16)
            nc.vector.tensor_copy(bqk_nat16[:], bqk_nat[:])
            b_qk = cpool.tile([128, H], F32)  # [p, m]

            # x0: split every s-tile into D-halves across both HWDGE queues
            xf_t[0] = xf_pool.tile([128, 5 * D], F32, tag="xf", name="xf")
            for si, (s0, psl) in enumerate(S_TILES):
                for hi, eng in ((0, nc.sync), (1, nc.scalar)):
                    eng.dma_start(
                        xf_t[0][0:psl, si * D + hi * 384 : si * D + (hi + 1) * 384],
                        x_ext[0, s0 : s0 + psl, hi * 384 : (hi + 1) * 384],
                    )
            emit_xT_cast(0)


            # qk-half k4,k5: contiguous f32 row-blocks on the (otherwise
            # idle) HWDGE queues + DVE cast (emitted after the xT copies so
            # the cast doesn't block the transpose psum-slot drains) -- takes
            # 2 of the 6 k-tiles off the serial SWDGE stream so scores pair 0
            # starts ~5us earlier
            with tc.tile_pool(name="wqstage", bufs=1) as wqs:
                wq_st = {}
                for i, k in enumerate((4, 5)):
                    wq_st[k] = wqs.tile(
                        [128, 2 * D], F32, tag=f"wq{i}", name=f"wq{i}"
                    )
                    eng = nc.sync if i == 0 else nc.scalar
                    eng.dma_start(
                        wq_st[k][:], wqkv_ext[k * 128 : (k + 1) * 128, 0 : 2 * D]
                    )

                for dk in range(NKT):
                    emit_xT_dk(0, dk)

                # broadcast v/fc biases to all 128 partitions via K=1 matmul
                b_v_bc = cpool.tile([128, D], F32)
                b_fc_bc = cpool.tile([128, D], F32)
                for row, bc in ((brow_v, b_v_bc), (brow_fc, b_fc_bc)):
                    for c0, cl in CH_D:
                        pb = psW.tile([128, 512], F32, tag="psW", name="pb")
                        nc.tensor.matmul(
                            pb[:, 0:cl],
                            lhsT=ones[0:1, 0:128],
                            rhs=row[0:1, c0 : c0 + cl],
                            start=True,
                            stop=True,
                        )
                        nc.vector.tensor_copy(bc[:, c0 : c0 + cl], pb[:, 0:cl])

                pbq = psW.tile([128, H], BF16, tag="psW", name="pbq")
                nc.tensor.transpose(
                    pbq[:, 0:H], bqk_nat16[0:H, 0:128], identity[0:H, 0:H]
                )
                nc.vector.tensor_copy(b_qk[:], pbq[:, 0:H])

                for k in (4, 5):
                    nc.vector.tensor_copy(w_qkv_k[k][:, 0 : 2 * D], wq_st[k][:])
            emit_x_dma(1)

            # ---- compute emission helpers ----
            qkT_t = {}

            qkT_pend = {}

            def emit_qkT_half(b, m, half):
                if half == 0:
                    if b not in qkT_t:
                        qkT_t[b] = {}
                    qkT_t[b][m] = qkT_pool.tile(
                        [128, S], BF16, tag=f"qkT{m}", name=f"qkT{m}"
                    )
                    qkT_pend[(b, m)] = {
                        0: psW.tile([128, 512], F32, tag="psW", name="pqk5"),
                        512: psW.tile([128, 65], F32, tag="psW", name="pqk65"),
                    }
                pqk = qkT_pend[(b, m)]
                ks = range(0, 3) if half == 0 else range(3, NKT)
                for k in ks:
                    for c0, cl in CH_S:
                        nc.tensor.matmul(
                            pqk[c0][:, 0:cl],
                            lhsT=w_qkv_k[k][:, m * 128 : (m + 1) * 128],
                            rhs=xT_t[b][k][:, c0 : c0 + cl],
                            start=(k == 0),
                            stop=(k == NKT - 1),
                        )
                if half == 1:
                    for c0, cl in CH_S:
                        nc.scalar.activation(
                            qkT_t[b][m][:, c0 : c0 + cl], pqk[c0][:, 0:cl],
                            IDENT, bias=b_qk[:, m : m + 1],
                        )
                    del qkT_pend[(b, m)]

            def emit_qkT_mtile(b, m):
                emit_qkT_half(b, m, 0)
                emit_qkT_half(b, m, 1)

            v_t = {}

            def emit_v_chunk(b, si, c0, cl):
                if b not in v_t:
                    v_t[b] = v_pool.tile(
                        [128, 5 * H * (DH + 1)], BF16, tag="v", name="v_all"
                    )
                    v4m = v_t[b][:].rearrange("p (s h e) -> p s h e", s=5, h=H)
                    nc.vector.memset(v4m[:, :, :, DH : DH + 1], 1.0)
                s0, psl = S_TILES[si]
                v4 = v_t[b][:].rearrange("p (s h e) -> p s h e", s=5, h=H)
                pv = psW.tile([128, cl], F32, tag="psW", name=f"pv{cl}")
                for k in range(NKT):
                    nc.tensor.matmul(
                        pv[0:psl, 0:cl],
                        lhsT=xT_t[b][k][:, s0 : s0 + psl],
                        rhs=w_qkv_k[k][:, 2 * D + c0 : 2 * D + c0 + cl],
                        start=(k == 0),
                        stop=(k == NKT - 1),
                    )
                h0, hn = c0 // DH, cl // DH
                nc.vector.tensor_add(
                    v4[0:psl, si, h0 : h0 + hn, 0:DH],
                    pv[0:psl, 0:cl].rearrange("p (h e) -> p h e", h=hn),
                    b_v_bc[0:psl, c0 : c0 + cl].rearrange("p (h e) -> p h e", h=hn),
                )

            def emit_v_si(b, si):
                for c0, cl in CH_D:
                    emit_v_chunk(b, si, c0, cl)

            def emit_scores_si(b, p, si, expT):
                heads = (2 * p, 2 * p + 1)
                s0, psl = S_TILES[si]
                psc = {}
                for h in heads:
                    if si == 0:
                        expT[h] = expT_pool.tile(
                            [128, 5 * S], BF16, tag="expT", name=f"expT{h % 2}"
                        )
                    psc[h] = psS.tile([128, S], F32, tag="psS", name=f"psc{h % 2}")
                for h in heads:
                    hoff = (h % 2) * 64
                    qm, km = h // 2, NKT + h // 2
                    for c0, cl in CH_S:
                        nc.tensor.matmul(
                            psc[h][0:psl, c0 : c0 + cl],
                            lhsT=qkT_t[b][km][hoff : hoff + 64, s0 : s0 + psl],
                            rhs=qkT_t[b][qm][hoff : hoff + 64, c0 : c0 + cl],
                            start=True,
                            stop=True,
                        )
                for h in heads:
                    nc.scalar.activation(
                        expT[h][0:psl, si * S : (si + 1) * S],
                        psc[h][0:psl, :],
                        EXP,
                        scale=float(SCALE),
                    )

            def emit_attnv_head(b, h, attnT_all, expT):
                hoff = (h % 2) * 64
                # attn_outT [65, 577]: rows 0:64 = out^T unnorm, row 64 = sums
                po = {0: psW.tile([65, 512], F32, tag="psW", name="po5"),
                      512: psW.tile([65, 65], F32, tag="psW", name="po65")}
                for si, (s0, psl) in enumerate(S_TILES):
                    for c0, cl in CH_S:
                        nc.tensor.matmul(
                            po[c0][:, 0:cl],
                            lhsT=v_t[b][
                                0:psl,
                                si * H * (DH + 1)
                                + h * (DH + 1) : si * H * (DH + 1)
                                + (h + 1) * (DH + 1),
                            ],
                            rhs=expT[h][0:psl, si * S + c0 : si * S + c0 + cl],
                            start=(si == 0),
                            stop=(si == 4),
                        )
                # normalize: sums row to SBUF (reciprocal can't read PSUM on
                # HW), reciprocal, broadcast, fused PSUM multiply
                rs = small_pool.tile([1, S], F32, tag="rs", name=f"rs{h % 2}")
                for c0, cl in CH_S:
                    nc.vector.tensor_copy(rs[:, c0 : c0 + cl], po[c0][64:65, 0:cl])
                rinv = small_pool.tile([1, S], F32, tag="rinv", name=f"rinv{h % 2}")
                nc.vector.reciprocal_approx_fast(rinv[:], rs[:])
                rbc = small_pool.tile([64, S], F32, tag="rbc")
                nc.gpsimd.partition_broadcast(rbc[:, :], rinv[0:1, :], channels=64)
                for c0, cl in CH_S:
                    nc.vector.tensor_mul(
                        attnT_all[
                            hoff : hoff + 64, (h // 2) * S + c0 : (h // 2) * S + c0 + cl
                        ],
                        po[c0][0:64, 0:cl],
                        rbc[:, c0 : c0 + cl],
                    )
                del expT[h]

            def emit_fc_chunk(b, si, c0, cl):
                s0, psl = S_TILES[si]
                attnT_all = attnT_t[b]
                pf = psW.tile([128, cl], F32, tag="psW", name=f"pf{cl}")
                for k in range(NKT):
                    nc.tensor.matmul(
                        pf[0:psl, 0:cl],
                        lhsT=attnT_all[:, k * S + s0 : k * S + s0 + psl],
                        rhs=w_fc_k[k][:, c0 : c0 + cl],
                        start=(k == 0),
                        stop=(k == NKT - 1),
                    )
                osb = osb_pool.tile([128, cl], F32, tag="osb")
                nc.vector.tensor_add(
                    osb[0:psl, 0:cl], pf[0:psl, 0:cl], b_fc_bc[0:psl, c0 : c0 + cl]
                )
                eng = nc.sync if (si + c0 // 512) % 2 else nc.scalar
                eng.dma_start(
                    out_ext[b, s0 : s0 + psl, c0 : c0 + cl], osb[0:psl, 0:cl]
                )

            def emit_fc_si(b, si):
                for c0, cl in CH_D:
                    emit_fc_chunk(b, si, c0, cl)

            # ---- prologue PE work: just enough to reach scores pair 0 ----
            emit_qkT_mtile(0, 0)
            emit_qkT_mtile(0, 6)

            # ---- deficit-tracked filler scheduler with deadline forcing ----
            PE_SCORES_SI = 0.45
            PE_ATTNV_HEAD = 1.30
            SC_EXP_SI = 1.06
            state = {"pe": 0.0, "sc": 0.0, "spent": 0.0}
            fill_q = []  # list of [key, cost_us, fn]

            def emit_unit(i):
                key, cost, fn = fill_q.pop(i)
                fn()
                state["pe"] += cost
                state["spent"] += cost

            def force(*keys):
                # emit every queued unit whose key prefix-matches (both
                # halves of a split unit), in queue order
                for key in keys:
                    while True:
                        idx = next(
                            (i for i, e in enumerate(fill_q)
                             if e[0][: len(key)] == key),
                            None,
                        )
                        if idx is None:
                            break
                        emit_unit(idx)

            attnT_t = {}
            for b in range(NB):
                if b + 2 < NB:
                    emit_x_dma(b + 2)
                # this batch's filler queue in DEADLINE order: this batch's
                # qkT, then next batch's qkT/v/xT (their DVE tails must land
                # before the next batch starts), fc last (no deadline)
                def u_qkT(bb, mm):
                    return [
                        [("qkT", bb, mm, 0), 0.72,
                         (lambda: emit_qkT_half(bb, mm, 0))],
                        [("qkT", bb, mm, 1), 0.73,
                         (lambda: emit_qkT_half(bb, mm, 1))],
                    ]

                def u_v(bb, ss):
                    return [
                        [("v", bb, ss, 0), 1.30,
                         (lambda: emit_v_chunk(bb, ss, 0, 512))],
                        [("v", bb, ss, 1), 0.66,
                         (lambda: emit_v_chunk(bb, ss, 512, 256))],
                    ]

                def u_fc(bb, ss):
                    return [
                        [("fc", bb, ss, 0), 1.35,
                         (lambda: emit_fc_chunk(bb, ss, 0, 512))],
                        [("fc", bb, ss, 1), 0.68,
                         (lambda: emit_fc_chunk(bb, ss, 512, 256))],
                    ]

                fq = []
                if b == 0:
                    fq.append([("xTc", 1), 0.0, lambda: emit_xT_cast(1)])
                    for dk in range(NKT):
                        fq.append(
                            [("xT", 1, dk), 0.75,
                             (lambda kk: lambda: emit_xT_dk(1, kk))(dk)]
                        )
                    for m in (1, 7):
                        fq.extend(u_qkT(0, m))
                    for si in range(5):
                        fq.extend(u_v(0, si))
                for p in range(2, NKT):
                    for m in (p, NKT + p):
                        fq.extend(u_qkT(b, m))
                if b + 1 < NB:
                    for m in (0, 6, 1, 7):
                        fq.extend(u_qkT(b + 1, m))
                    for si in range(5):
                        fq.extend(u_v(b + 1, si))
                if b + 2 < NB:
                    fq.append(
                        [("xTc", b + 2), 0.0,
                         (lambda bb: lambda: emit_xT_cast(bb))(b + 2)]
                    )
                    for dk in range(NKT):
                        fq.append(
                            [("xT", b + 2, dk), 0.75,
                             (lambda bb, kk: lambda: emit_xT_dk(bb, kk))(b + 2, dk)]
                        )
                if b >= 1:
                    for si in range(5):
                        fq.extend(u_fc(b - 1, si))
                fill_q = fq
                total_cost = sum(e[1] for e in fq)
                state["pe"] = 0.0
                state["sc"] = 0.0
                state["spent"] = 0.0
                point = [0]
                N_POINTS = 5 * (H // 2) + H  # 30 si-groups + 12 attnv heads

                def quota_fill():
                    # guaranteed even spreading: by point i, (i+1)/N of the
                    # queue's cost must be emitted, with at least one unit per
                    # point (sub-us stalls trip the HAM clock-gate window);
                    # plus the exp-deficit floor
                    point[0] += 1
                    tgt = total_cost * point[0] / N_POINTS
                    first = len(fill_q) > (N_POINTS - point[0])
                    while fill_q and (
                        first
                        or state["spent"] < tgt
                        or state["pe"] < state["sc"] + 0.4
                    ):
                        first = False
                        emit_unit(0)

                attnT_t[b] = attnT_pool.tile(
                    [128, NKT * S], BF16, tag="attnT", name="attnT_all"
                )
                # deadline safety: v + first qkT tiles must exist before use
                if b > 0:
                    force(("qkT", b, 0), ("qkT", b, 6), ("qkT", b, 1), ("qkT", b, 7),
                          *((("v", b, si)) for si in range(5)))
                expT = {}
                for p in range(H // 2 + 1):
                    if p < H // 2:
                        force(("qkT", b, p), ("qkT", b, NKT + p))
                        for si in range(5):
                            emit_scores_si(b, p, si, expT)
                            state["pe"] += PE_SCORES_SI
                            state["sc"] += SC_EXP_SI
                            quota_fill()
                    if p >= 1:
                        if p == 1:
                            force(*[("v", b, si) for si in range(5)])
                        for h in (2 * (p - 1), 2 * (p - 1) + 1):
                            emit_attnv_head(b, h, attnT_t[b], expT)
                            state["pe"] += PE_ATTNV_HEAD
                            quota_fill()
                # drain leftovers (should be nearly empty now)
                while fill_q:
                    emit_unit(0)

            for si in range(5):
                emit_fc_si(NB - 1, si)

    nc.compile()
    return nc


_NC_CACHE = None


def _get_nc():
    global _NC_CACHE
    if _NC_CACHE is None:
        _NC_CACHE = build_nc()
    return _NC_CACHE


def kernel(x, w_qkv, b_qkv, w_fc, b_fc, _collect=None):
    nc = _get_nc()
    x = np.ascontiguousarray(np.asarray(x, dtype=np.float32))
    w_qkv = np.ascontiguousarray(np.asarray(w_qkv, dtype=np.float32))
    b_qkv = np.ascontiguousarray(np.asarray(b_qkv, dtype=np.float32))
    w_fc = np.ascontiguousarray(np.asarray(w_fc, dtype=np.float32))
    b_fc = np.ascontiguousarray(np.asarray(b_fc, dtype=np.float32))
    in_maps = [
        {
            "x": x[i * NB : (i + 1) * NB],
            "w_qkv": w_qkv,
            "b_qkv": b_qkv,
            "w_fc": w_fc,
            "b_fc": b_fc,
        }
        for i in range(NCORES)
    ]
    kwargs = dict(_collect) if _collect else {}
    res = run_bass_kernel_spmd(nc, in_maps, core_ids=list(range(NCORES)), **kwargs)
    out = np.concatenate([res.results[i]["out"] for i in range(NCORES)], axis=0)
    if _collect is not None and isinstance(_collect, dict):
        _collect["result"] = res
    return out.astype(np.float32)


if __name__ == "__main__":
    xs = np.random.randn(B, S, D).astype(np.float32)
    lim = 1.0 / np.sqrt(D)
    rng = np.random.default_rng(0)
    wq = rng.uniform(-lim, lim, (D, 3 * D)).astype(np.float32)
    bq = rng.uniform(-lim, lim, (3 * D,)).astype(np.float32)
    wf = rng.uniform(-lim, lim, (D, D)).astype(np.float32)
    bf = rng.uniform(-lim, lim, (D,)).astype(np.float32)
    o = kernel(xs, wq, bq, wf, bf)
    print("out", o.shape, o.dtype)


# revision 51
# speedup vs baseline: 1.0092x; 1.0073x over previous
"""Trainium2 Bass kernel: 12-head attention block (qkv proj -> softmax attn -> fc).

Reference semantics (B=32, S=577, D=768, H=12, Dh=64):
    qkv = x @ w_qkv + b_qkv
    q, k, v = split(qkv); attn = softmax(q k^T / 8) v
    out = attn @ w_fc + b_fc

Sharding: data-parallel over batch across 8 NeuronCores (4 images per core),
weights replicated, no collectives. Compute in bf16 with fp32 PSUM accumulation.

Layout (all matmuls contract over the partition dim):
  - xT [768, 577] per image: x lands f32 via the two HWDGE queues (each
    s-tile split into D-halves across both), one DVE cast to bf16 per
    s-tile, then PE transposes in bf16 (1 cyc/col + fast weight load; the
    old fp32 transpose-mode path ran at 1/4 rate).
  - qkT [1536, 577] = w_qkv[:, :1536]^T . xT; per-partition qkv bias is
    loaded as a contiguous [12, 128] block and PE-transposed (the direct
    [p, m] gather is 1536 4B descriptors and camps on the sync queue).
  - v [577, 768] + per-head ones column so attention row-sums fall out of
    the attn@v matmul for free.
  - scoresT[sk, sq] = kT_h^T . qT_h; head pairs on disjoint PE row groups;
    exp on ScalarE (scale folded in; both chunks of a head emitted
    back-to-back so its psum bank frees as early as possible).
  - attn_outT[65, sq]: row 64 = softmax denominators; normalize =
    rs copy -> reciprocal_approx_fast (NOT from PSUM: silently wrong on HW)
    -> gpsimd partition_broadcast [64 rows] -> fused DVE multiply straight
    from PSUM into attnT (saves a full unnormalized-copy pass).
  - fc: out[s, :] = attnT_k^T . w_fc_k; per-(si, 512/256-chunk) units with
    per-chunk output DMAs round-robined over both HWDGE queues.

Scheduling (the part that matters): the attention phase is paced by exp on
ScalarE (~1.06us per scores si-group vs ~0.45us of PE), so all other PE
work -- qkT half-chains, v/fc chunks, next image's transposes -- is kept in
a deadline-ordered queue of ~0.7-1.3us units and injected between scores
si-groups by a quota + exp-deficit scheduler.  Sub-us PE idle gaps are not
just lost time: enough of them in a 3.4us window re-throttles the PE HAM
clock gate to half rate, so even spreading matters more than total filler
supply (the old one-filler-per-si-group scheme starved pairs 3-5 and ran
13-20us of every batch at 1.2GHz).  Every psum tile is a single 1-bank
chunk (psW pool bufs=4) so a chain's first matmul only waits on a DVE
drain 4 allocations back.  Weight DMAs are contiguous row-blocks only
(SWDGE casting throughput collapses ~7x on column-sliced reads): q/k half
of k0-3 + v half + w_fc on the SWDGE casting queue, q/k of k4-5 staged f32
on the idle HWDGE queues and DVE-cast, so scores pair 0 issues ~5us
earlier than a pure-SWDGE stream allows.
PSUM: psS (scores) 2x2-bank slots, psW (everything else) 4x1-bank slots.
"""

import os
import sys

import numpy as np

for _p in ("/opt/trn_rl_repo", "/root/.axon_site/_ro/trn_rl_repo"):
    if os.path.isdir(_p) and _p not in sys.path:
        sys.path.insert(0, _p)

import concourse.bass as bass  # noqa: E402
import concourse.tile as tile  # noqa: E402
from concourse import bacc, mybir  # noqa: E402
from concourse.bass_utils import run_bass_kernel_spmd  # noqa: E402
from concourse.masks import make_identity  # noqa: E402

F32 = mybir.dt.float32
BF16 = mybir.dt.bfloat16

B, S, D = 32, 577, 768
H, DH = 12, 64
NCORES = 8
NB = B // NCORES  # 4 batch images per core
SCALE = DH**-0.5  # 0.125
NKT = D // 128  # 6 contraction tiles of 128
S_TILES = [(0, 128), (128, 128), (256, 128), (384, 128), (512, 65)]
CH_S = [(0, 512), (512, 65)]  # 577 split at PSUM-bank boundary
CH_D = [(0, 512), (512, 256)]  # 768 split at PSUM-bank boundary
SPAD = 640  # xT column allocation (577 rounded up to 5*128)
EXP = mybir.ActivationFunctionType.Exp
IDENT = mybir.ActivationFunctionType.Identity


def build_nc():
    nc = bacc.Bacc(None)
    x_ext = nc.declare_dram_parameter("x", [NB, S, D], F32, isOutput=False)
    wqkv_ext = nc.declare_dram_parameter("w_qkv", [D, 3 * D], F32, isOutput=False)
    bqkv_ext = nc.declare_dram_parameter("b_qkv", [3 * D], F32, isOutput=False)
    wfc_ext = nc.declare_dram_parameter("w_fc", [D, D], F32, isOutput=False)
    bfc_ext = nc.declare_dram_parameter("b_fc", [D], F32, isOutput=False)
    out_ext = nc.declare_dram_parameter("out", [NB, S, D], F32, isOutput=True)

    with tile.TileContext(nc) as tc:
        with (
            tc.tile_pool(name="const", bufs=1) as cpool,
            tc.tile_pool(name="xf", bufs=1) as xf_pool,
            tc.tile_pool(name="xb", bufs=1) as xb_pool,
            tc.tile_pool(name="xT", bufs=2) as xT_pool,
            tc.tile_pool(name="qkT", bufs=2) as qkT_pool,
            tc.tile_pool(name="v", bufs=2) as v_pool,
            tc.tile_pool(name="expT", bufs=4) as expT_pool,
            tc.tile_pool(name="attnT", bufs=2) as attnT_pool,
            tc.tile_pool(name="small", bufs=3) as small_pool,
            tc.tile_pool(name="osb", bufs=3) as osb_pool,
            tc.tile_pool(name="psS", bufs=2, space="PSUM") as psS,
            tc.tile_pool(name="psW", bufs=4, space="PSUM") as psW,
        ):
            # ---- small constants first on the sync HWDGE (x0 right behind) ----
            ones = cpool.tile([1, 128], F32)
            nc.vector.memset(ones[:], 1.0)
            identity = cpool.tile([128, 128], BF16)
            make_identity(nc, identity[:])
            brow_v = cpool.tile([1, D], F32)
            nc.sync.dma_start(brow_v[:], bqkv_ext[None, 2 * D : 3 * D])
            brow_fc = cpool.tile([1, D], F32)
            nc.scalar.dma_start(brow_fc[:], bfc_ext[None, :])

            # ---- x pipeline: f32 DMA (HWDGE) -> DVE cast -> XBAR transpose ----
            xf_t, xb_t, xT_t = {}, {}, {}

            def emit_x_dma(b):
                xf_t[b] = xf_pool.tile([128, 5 * D], F32, tag="xf", name="xf")
                for si, (s0, psl) in enumerate(S_TILES):
                    eng = nc.sync if si % 2 == 0 else nc.scalar
                    eng.dma_start(
                        xf_t[b][0:psl, si * D : (si + 1) * D],
                        x_ext[b, s0 : s0 + psl, :],
                    )

            def emit_xT_cast(b):
                # cast each s-tile to bf16 (one DVE copy per s-tile)
                xb_t[b] = xb_pool.tile([128, 5 * D], BF16, tag="xb", name="xb")
                xT_t[b] = {}
                for si, (s0, psl) in enumerate(S_TILES):
                    nc.vector.tensor_copy(
                        xb_t[b][0:psl, si * D : (si + 1) * D],
                        xf_t[b][0:psl, si * D : (si + 1) * D],
                    )

            def emit_xT_dk(b, dk):
                # PE-transpose one 128-row k-tile in bf16 (1 cyc/col + fast
                # weight load, vs 1/4 rate in f32)
                xT_t[b][dk] = xT_pool.tile(
                    [128, S], BF16, tag=f"xT{dk}", name=f"xT{dk}"
                )
                px = psW.tile([128, S], BF16, tag="psW", name="px")
                for si, (s0, psl) in enumerate(S_TILES):
                    nc.tensor.transpose(
                        px[:, s0 : s0 + psl],
                        xb_t[b][0:psl, si * D + dk * 128 : si * D + (dk + 1) * 128],
                        identity[0:psl, 0:psl],
                    )
                nc.vector.tensor_copy(xT_t[b][dk][:], px[:])

            def emit_xT(b):
                emit_xT_cast(b)
                for dk in range(NKT):
                    emit_xT_dk(b, dk)

            # ---- weight tiles (SBUF, bf16, SWDGE casting DMAs) ----
            # contiguous row-block DMAs only: SWDGE casting throughput
            # collapses ~7x on column-sliced (strided) reads
            w_qkv_k = [
                cpool.tile([128, 3 * D], BF16, name=f"wqkv{k}") for k in range(NKT)
            ]
            w_fc_k = [cpool.tile([128, D], BF16, name=f"wfc{k}") for k in range(NKT)]
            for k in range(4):
                # q/k half for k4,k5 arrives via HWDGE f32 + DVE cast below
                nc.gpsimd.dma_start(
                    w_qkv_k[k][:, 0 : 2 * D],
                    wqkv_ext[k * 128 : (k + 1) * 128, 0 : 2 * D],
                )
            for k in range(NKT):
                nc.gpsimd.dma_start(
                    w_qkv_k[k][:, 2 * D : 3 * D],
                    wqkv_ext[k * 128 : (k + 1) * 128, 2 * D : 3 * D],
                )
            for k in range(NKT):
                nc.gpsimd.dma_start(w_fc_k[k][:], wfc_ext[k * 128 : (k + 1) * 128, :])

            # per-partition bias for qkT tiles: load [12, 128] contiguous and
            # PE-transpose (the direct [p, m] gather is 1536 4B descriptors
            # and occupies the sync queue for ~30us)
            bqk_nat = cpool.tile([12, 128], F32)
            nc.sync.dma_start(
                bqk_nat[:], bqkv_ext[0 : 2 * D].rearrange("(m p) -> m p", p=128)
            )
            bqk_nat16 = cpool.tile([12, 128], BF16)
            nc.vector.tensor_copy(bqk_nat16[:], bqk_nat[:])
            b_qk = cpool.tile([128, H], F32)  # [p, m]

            # x0: split every s-tile into D-halves across both HWDGE queues
            xf_t[0] = xf_pool.tile([128, 5 * D], F32, tag="xf", name="xf")
            for si, (s0, psl) in enumerate(S_TILES):
                for hi, eng in ((0, nc.sync), (1, nc.scalar)):
                    eng.dma_start(
                        xf_t[0][0:psl, si * D + hi * 384 : si * D + (hi + 1) * 384],
                        x_ext[0, s0 : s0 + psl, hi * 384 : (hi + 1) * 384],
                    )
            emit_xT_cast(0)


            # qk-half k4,k5: contiguous f32 row-blocks on the (otherwise
            # idle) HWDGE queues + DVE cast (emitted after the xT copies so
            # the cast doesn't block the transpose psum-slot drains) -- takes
            # 2 of the 6 k-tiles off the serial SWDGE stream so scores pair 0
            # starts ~5us earlier
            with tc.tile_pool(name="wqstage", bufs=1) as wqs:
                wq_st = {}
                for i, k in enumerate((4, 5)):
                    wq_st[k] = wqs.tile(
                        [128, 2 * D], F32, tag=f"wq{i}", name=f"wq{i}"
                    )
                    eng = nc.sync if i == 0 else nc.scalar
                    eng.dma_start(
                        wq_st[k][:], wqkv_ext[k * 128 : (k + 1) * 128, 0 : 2 * D]
                    )

                for dk in range(NKT):
                    emit_xT_dk(0, dk)

                # broadcast v/fc biases to all 128 partitions via K=1 matmul
                b_v_bc = cpool.tile([128, D], F32)
                b_fc_bc = cpool.tile([128, D], F32)
                for row, bc in ((brow_v, b_v_bc), (brow_fc, b_fc_bc)):
                    for c0, cl in CH_D:
                        pb = psW.tile([128, 512], F32, tag="psW", name="pb")
                        nc.tensor.matmul(
                            pb[:, 0:cl],
                            lhsT=ones[0:1, 0:128],
                            rhs=row[0:1, c0 : c0 + cl],
                            start=True,
                            stop=True,
                        )
                        nc.vector.tensor_copy(bc[:, c0 : c0 + cl], pb[:, 0:cl])

                pbq = psW.tile([128, H], BF16, tag="psW", name="pbq")
                nc.tensor.transpose(
                    pbq[:, 0:H], bqk_nat16[0:H, 0:128], identity[0:H, 0:H]
                )
                nc.vector.tensor_copy(b_qk[:], pbq[:, 0:H])

                for k in (4, 5):
                    nc.vector.tensor_copy(w_qkv_k[k][:, 0 : 2 * D], wq_st[k][:])
            emit_x_dma(1)

            # ---- compute emission helpers ----
            qkT_t = {}

            qkT_pend = {}

            def emit_qkT_half(b, m, half):
                if half == 0:
                    if b not in qkT_t:
                        qkT_t[b] = {}
                    qkT_t[b][m] = qkT_pool.tile(
                        [128, S], BF16, tag=f"qkT{m}", name=f"qkT{m}"
                    )
                    qkT_pend[(b, m)] = {
                        0: psW.tile([128, 512], F32, tag="psW", name="pqk5"),
                        512: psW.tile([128, 65], F32, tag="psW", name="pqk65"),
                    }
                pqk = qkT_pend[(b, m)]
                ks = range(0, 3) if half == 0 else range(3, NKT)
                for k in ks:
                    for c0, cl in CH_S:
                        nc.tensor.matmul(
                            pqk[c0][:, 0:cl],
                            lhsT=w_qkv_k[k][:, m * 128 : (m + 1) * 128],
                            rhs=xT_t[b][k][:, c0 : c0 + cl],
                            start=(k == 0),
                            stop=(k == NKT - 1),
                        )
                if half == 1:
                    for c0, cl in CH_S:
                        nc.scalar.activation(
                            qkT_t[b][m][:, c0 : c0 + cl], pqk[c0][:, 0:cl],
                            IDENT, bias=b_qk[:, m : m + 1],
                        )
                    del qkT_pend[(b, m)]

            def emit_qkT_mtile(b, m):
                emit_qkT_half(b, m, 0)
                emit_qkT_half(b, m, 1)

            v_t = {}

            def emit_v_chunk(b, si, c0, cl):
                if b not in v_t:
                    v_t[b] = v_pool.tile(
                        [128, 5 * H * (DH + 1)], BF16, tag="v", name="v_all"
                    )
                    v4m = v_t[b][:].rearrange("p (s h e) -> p s h e", s=5, h=H)
                    nc.vector.memset(v4m[:, :, :, DH : DH + 1], 1.0)
                s0, psl = S_TILES[si]
                v4 = v_t[b][:].rearrange("p (s h e) -> p s h e", s=5, h=H)
                pv = psW.tile([128, cl], F32, tag="psW", name=f"pv{cl}")
                for k in range(NKT):
                    nc.tensor.matmul(
                        pv[0:psl, 0:cl],
                        lhsT=xT_t[b][k][:, s0 : s0 + psl],
                        rhs=w_qkv_k[k][:, 2 * D + c0 : 2 * D + c0 + cl],
                        start=(k == 0),
                        stop=(k == NKT - 1),
                    )
                h0, hn = c0 // DH, cl // DH
                nc.vector.tensor_add(
                    v4[0:psl, si, h0 : h0 + hn, 0:DH],
                    pv[0:psl, 0:cl].rearrange("p (h e) -> p h e", h=hn),
                    b_v_bc[0:psl, c0 : c0 + cl].rearrange("p (h e) -> p h e", h=hn),
                )

            def emit_v_si(b, si):
                for c0, cl in CH_D:
                    emit_v_chunk(b, si, c0, cl)

            def emit_scores_si(b, p, si, expT):
                heads = (2 * p, 2 * p + 1)
                s0, psl = S_TILES[si]
                psc = {}
                for h in heads:
                    if si == 0:
                        expT[h] = expT_pool.tile(
                            [128, 5 * S], BF16, tag="expT", name=f"expT{h % 2}"
                        )
                    psc[h] = psS.tile([128, S], F32, tag="psS", name=f"psc{h % 2}")
                for h in heads:
                    hoff = (h % 2) * 64
                    qm, km = h // 2, NKT + h // 2
                    for c0, cl in CH_S:
                        nc.tensor.matmul(
                            psc[h][0:psl, c0 : c0 + cl],
                            lhsT=qkT_t[b][km][hoff : hoff + 64, s0 : s0 + psl],
                            rhs=qkT_t[b][qm][hoff : hoff + 64, c0 : c0 + cl],
                            start=True,
                            stop=True,
                        )
                for h in heads:
                    nc.scalar.activation(
                        expT[h][0:psl, si * S : (si + 1) * S],
                        psc[h][0:psl, :],
                        EXP,
                        scale=float(SCALE),
                    )

            def emit_attnv_head(b, h, attnT_all, expT):
                hoff = (h % 2) * 64
                # attn_outT [65, 577]: rows 0:64 = out^T unnorm, row 64 = sums
                po = {0: psW.tile([65, 512], F32, tag="psW", name="po5"),
                      512: psW.tile([65, 65], F32, tag="psW", name="po65")}
                for si, (s0, psl) in enumerate(S_TILES):
                    for c0, cl in CH_S:
                        nc.tensor.matmul(
                            po[c0][:, 0:cl],
                            lhsT=v_t[b][
                                0:psl,
                                si * H * (DH + 1)
                                + h * (DH + 1) : si * H * (DH + 1)
                                + (h + 1) * (DH + 1),
                            ],
                            rhs=expT[h][0:psl, si * S + c0 : si * S + c0 + cl],
                            start=(si == 0),
                            stop=(si == 4),
                        )
                # normalize: sums row to SBUF (reciprocal can't read PSUM on
                # HW), reciprocal, broadcast, fused PSUM multiply
                rs = small_pool.tile([1, S], F32, tag="rs", name=f"rs{h % 2}")
                for c0, cl in CH_S:
                    nc.vector.tensor_copy(rs[:, c0 : c0 + cl], po[c0][64:65, 0:cl])
                rinv = small_pool.tile([1, S], F32, tag="rinv", name=f"rinv{h % 2}")
                nc.vector.reciprocal_approx_fast(rinv[:], rs[:])
                rbc = small_pool.tile([64, S], F32, tag="rbc")
                nc.gpsimd.partition_broadcast(rbc[:, :], rinv[0:1, :], channels=64)
                for c0, cl in CH_S:
                    nc.vector.tensor_mul(
                        attnT_all[
                            hoff : hoff + 64, (h // 2) * S + c0 : (h // 2) * S + c0 + cl
                        ],
                        po[c0][0:64, 0:cl],
                        rbc[:, c0 : c0 + cl],
                    )
                del expT[h]

            def emit_fc_chunk(b, si, c0, cl):
                s0, psl = S_TILES[si]
                attnT_all = attnT_t[b]
                pf = psW.tile([128, cl], F32, tag="psW", name=f"pf{cl}")
                for k in range(NKT):
                    nc.tensor.matmul(
                        pf[0:psl, 0:cl],
                        lhsT=attnT_all[:, k * S + s0 : k * S + s0 + psl],
                        rhs=w_fc_k[k][:, c0 : c0 + cl],
                        start=(k == 0),
                        stop=(k == NKT - 1),
                    )
                osb = osb_pool.tile([128, cl], F32, tag="osb")
                nc.vector.tensor_add(
                    osb[0:psl, 0:cl], pf[0:psl, 0:cl], b_fc_bc[0:psl, c0 : c0 + cl]
                )
                eng = nc.sync if (si + c0 // 512) % 2 else nc.scalar
                eng.dma_start(
                    out_ext[b, s0 : s0 + psl, c0 : c0 + cl], osb[0:psl, 0:cl]
                )

            def emit_fc_si(b, si):
                for c0, cl in CH_D:
                    emit_fc_chunk(b, si, c0, cl)

            # ---- prologue PE work: just enough to reach scores pair 0 ----
            emit_qkT_mtile(0, 0)
            emit_qkT_mtile(0, 6)

            # ---- deficit-tracked filler scheduler with deadline forcing ----
            PE_SCORES_SI = 0.45
            PE_ATTNV_HEAD = 1.30
            SC_EXP_SI = 1.06
            state = {"pe": 0.0, "sc": 0.0, "spent": 0.0}
            fill_q = []  # list of [key, cost_us, fn]

            def emit_unit(i):
                key, cost, fn = fill_q.pop(i)
                fn()
                state["pe"] += cost
                state["spent"] += cost

            def force(*keys):
                # emit every queued unit whose key prefix-matches (both
                # halves of a split unit), in queue order
                for key in keys:
                    while True:
                        idx = next(
                            (i for i, e in enumerate(fill_q)
                             if e[0][: len(key)] == key),
                            None,
                        )
                        if idx is None:
                            break
                        emit_unit(idx)

            attnT_t = {}
            for b in range(NB):
                if b + 2 < NB:
                    emit_x_dma(b + 2)
                # this batch's filler queue in DEADLINE order: this batch's
                # qkT, then next batch's qkT/v/xT (their DVE tails must land
                # before the next batch starts), fc last (no deadline)
                def u_qkT(bb, mm):
                    return [
                        [("qkT", bb, mm, 0), 0.72,
                         (lambda: emit_qkT_half(bb, mm, 0))],
                        [("qkT", bb, mm, 1), 0.73,
                         (lambda: emit_qkT_half(bb, mm, 1))],
                    ]

                def u_v(bb, ss):
                    return [
                        [("v", bb, ss, 0), 1.30,
                         (lambda: emit_v_chunk(bb, ss, 0, 512))],
                        [("v", bb, ss, 1), 0.66,
                         (lambda: emit_v_chunk(bb, ss, 512, 256))],
                    ]

                def u_fc(bb, ss):
                    return [
                        [("fc", bb, ss, 0), 1.35,
                         (lambda: emit_fc_chunk(bb, ss, 0, 512))],
                        [("fc", bb, ss, 1), 0.68,
                         (lambda: emit_fc_chunk(bb, ss, 512, 256))],
                    ]

                fq = []
                if b == 0:
                    fq.append([("xTc", 1), 0.0, lambda: emit_xT_cast(1)])
                    for dk in range(NKT):
                        fq.append(
                            [("xT", 1, dk), 0.75,
                             (lambda kk: lambda: emit_xT_dk(1, kk))(dk)]
                        )
                    for m in (1, 7):
                        fq.extend(u_qkT(0, m))
                    for si in range(5):
                        fq.extend(u_v(0, si))
                for p in range(2, NKT):
                    for m in (p, NKT + p):
                        fq.extend(u_qkT(b, m))
                if b + 1 < NB:
                    for m in (0, 6, 1, 7):
                        fq.extend(u_qkT(b + 1, m))
                    for si in range(5):
                        fq.extend(u_v(b + 1, si))
                if b + 2 < NB:
                    fq.append(
                        [("xTc", b + 2), 0.0,
                         (lambda bb: lambda: emit_xT_cast(bb))(b + 2)]
                    )
                    for dk in range(NKT):
                        fq.append(
                            [("xT", b + 2, dk), 0.75,
                             (lambda bb, kk: lambda: emit_xT_dk(bb, kk))(b + 2, dk)]
                        )
                if b >= 1:
                    for si in range(5):
                        fq.extend(u_fc(b - 1, si))
                fill_q = fq
                total_cost = sum(e[1] for e in fq)
                state["pe"] = 0.0
                state["sc"] = 0.0
                state["spent"] = 0.0
                point = [0]
                N_POINTS = 5 * (H // 2) + H  # 30 si-groups + 12 attnv heads

                def quota_fill():
                    # guaranteed even spreading: by point i, (i+1)/N of the
                    # queue's cost must be emitted, with at least one unit per
                    # point (sub-us stalls trip the HAM clock-gate window);
                    # plus the exp-deficit floor
                    point[0] += 1
                    tgt = total_cost * point[0] / N_POINTS
                    first = len(fill_q) > (N_POINTS - point[0])
                    while fill_q and (
                        first
                        or state["spent"] < tgt
                        or state["pe"] < state["sc"] + 0.4
                    ):
                        first = False
                        emit_unit(0)

                attnT_t[b] = attnT_pool.tile(
                    [128, NKT * S], BF16, tag="attnT", name="attnT_all"
                )
                # deadline safety: v + first qkT tiles must exist before use
                if b > 0:
                    force(("qkT", b, 0), ("qkT", b, 6), ("qkT", b, 1), ("qkT", b, 7),
                          *((("v", b, si)) for si in range(5)))
                expT = {}
                for p in range(H // 2 + 1):
                    if p < H // 2:
                        force(("qkT", b, p), ("qkT", b, NKT + p))
                        for si in range(5):
                            emit_scores_si(b, p, si, expT)
                            state["pe"] += PE_SCORES_SI
                            state["sc"] += SC_EXP_SI
                            quota_fill()
                    if p >= 1:
                        if p == 1:
                            force(*[("v", b, si) for si in range(5)])
                        for h in (2 * (p - 1), 2 * (p - 1) + 1):
                            emit_attnv_head(b, h, attnT_t[b], expT)
                            state["pe"] += PE_ATTNV_HEAD
                            quota_fill()
                # drain leftovers (should be nearly empty now)
                while fill_q:
                    emit_unit(0)

            for si in range(5):
                emit_fc_si(NB - 1, si)

    nc.compile()
    return nc


_NC_CACHE = None


def _get_nc():
    global _NC_CACHE
    if _NC_CACHE is None:
        _NC_CACHE = build_nc()
    return _NC_CACHE


def kernel(x, w_qkv, b_qkv, w_fc, b_fc, _collect=None):
    nc = _get_nc()
    x = np.ascontiguousarray(np.asarray(x, dtype=np.float32))
    w_qkv = np.ascontiguousarray(np.asarray(w_qkv, dtype=np.float32))
    b_qkv = np.ascontiguousarray(np.asarray(b_qkv, dtype=np.float32))
    w_fc = np.ascontiguousarray(np.asarray(w_fc, dtype=np.float32))
    b_fc = np.ascontiguousarray(np.asarray(b_fc, dtype=np.float32))
    in_maps = [
        {
            "x": x[i * NB : (i + 1) * NB],
            "w_qkv": w_qkv,
            "b_qkv": b_qkv,
            "w_fc": w_fc,
            "b_fc": b_fc,
        }
        for i in range(NCORES)
    ]
    kwargs = dict(_collect) if _collect else {}
    res = run_bass_kernel_spmd(nc, in_maps, core_ids=list(range(NCORES)), **kwargs)
    out = np.concatenate([res.results[i]["out"] for i in range(NCORES)], axis=0)
    if _collect is not None and isinstance(_collect, dict):
        _collect["result"] = res
    return out.astype(np.float32)


if __name__ == "__main__":
    xs = np.random.randn(B, S, D).astype(np.float32)
    lim = 1.0 / np.sqrt(D)
    rng = np.random.default_rng(0)
    wq = rng.uniform(-lim, lim, (D, 3 * D)).astype(np.float32)
    bq = rng.uniform(-lim, lim, (3 * D,)).astype(np.float32)
    wf = rng.uniform(-lim, lim, (D, D)).astype(np.float32)
    bf = rng.uniform(-lim, lim, (D,)).astype(np.float32)
    o = kernel(xs, wq, bq, wf, bf)
    print("out", o.shape, o.dtype)
